# revision 28
# baseline (speedup 1.0000x reference)
"""Trainium2 Bass kernel for a GPT-2 style transformer block (pre-LN, causal
attention, tanh-GELU MLP), distributed over 8 NeuronCores.

Sharding: HEAD-parallel attention + token-parallel MLP.
Core c (b = c//4, r = c%4) computes, for sequence b:
  - LN1 + Q/K/V for its 4 heads over the FULL 2048-token sequence,
  - exact-causal attention for those 4 heads,
  - its cproj PARTIAL (contracting its 256 y-features) for all 2048 tokens,
  - a 4-core ReduceScatter per 512-token chunk hands each core its own
    128-token stripe of that chunk,
  - residual + LN2 + MLP + residual for its 4 stripes (token-parallel).

Pipeline structure (PE-dense by construction; the in-order tensor queue is
never parked behind a cross-engine dependency chain):
  Phase A: per 128-token tile: LN1 stats/apply -> PE transposes -> V matmul;
           after every 4th tile, that 512-chunk's Q/K matmuls. The DVE/ACT
           LN work for tile t+1 runs under tile t's PE work.
  Phase B: attention per 512-query chunk (ascending): per key tile, both
           heads' scores run CONCURRENTLY via 64-row PE tiling; exp on ACT;
           PV lags scores by one key tile so PE never waits on ACT.
           Softmax denominators ride in V as a ones column (weights 0 +
           bias 1); odd heads carry the ones column FIRST and their PV
           output lands at PSUM partitions 63..127, so y sits at 64:128 and
           the per-query reciprocal (DVE) + partition_broadcast (GpSimd)
           feed partition-aligned multiplies - no DMA in the chain.
           cproj consumes f2=0 then f2=1 (f2=1's normalize finishes under
           the f2=0 half-chain); ReduceScatter per chunk right after.
  Phase C: MLP per stripe-PAIR {0,1} then {2,3}: LN2 + fc (N=256) + mproj,
           so fc starts once the second RS lands instead of the fourth,
           and the later RSs hide under the first pair's fc/mproj.

All per-core variation (which heads, which stripes) is carried in input
DATA so the SPMD graph is identical on every core.
"""

import math
from contextlib import ExitStack

import numpy as np
import ml_dtypes

import concourse.bass as bass
import concourse.mybir as mybir
import concourse.tile as tile
from concourse import bacc
from concourse.masks import make_identity

F32 = mybir.dt.float32
BF16 = mybir.dt.bfloat16
AF = mybir.ActivationFunctionType

N_CORES = 8
B, T, C = 2, 2048, 1024
H = 16
HD = C // H            # 64
HL = 4                 # heads per core
FF = 4 * C             # 4096
CHUNK = 512            # tokens owned per core for MLP/output
GRP = 4                # cores per sequence
P = 128
NC_T = C // P          # 8
NT_SEQ = T // P        # 16
NT_OWN = CHUNK // P    # 4
NQC = T // 512         # 4 query chunks
EPS = 1e-5
VW = HL * (HD + 1)     # 260: V row width incl. per-head ones column
WQKV_W = 2 * HL * HD + VW  # 772
REPLICA_GROUPS = [[0, 1, 2, 3], [4, 5, 6, 7]]


def _ln_tile(nc, stats_pool, x_tile, out_bf, eps_t):
    """LayerNorm stats+apply for one [P, C] f32 tile -> bf16 xhat tile."""
    stats = stats_pool.tile([P, 2, 6], F32, tag="stats", name="stats")
    mv = stats_pool.tile([P, 2], F32, tag="mv", name="mv")
    x3 = x_tile.rearrange("p (s d) -> p s d", s=2)
    for s in range(2):
        nc.vector.bn_stats(stats[:, s, :], x3[:, s, :])
    nc.vector.bn_aggr(mv[:], stats[:])
    rstd = stats_pool.tile([P, 1], F32, tag="rstd", name="rstd")
    nc.scalar.activation(rstd[:], mv[:, 1:2], AF.Sqrt, bias=eps_t[:],
                         scale=1.0)
    nc.vector.reciprocal(rstd[:], rstd[:])
    # xhat = x * rstd + (-mu * rstd), applied on the ACT engine
    nmr = stats_pool.tile([P, 1], F32, tag="nmr", name="nmr")
    nc.vector.tensor_scalar(
        out=nmr[:], in0=mv[:, 0:1], scalar1=rstd[:], scalar2=-1.0,
        op0=mybir.AluOpType.mult, op1=mybir.AluOpType.mult)
    nc.scalar.activation(out_bf[:], x_tile[:], AF.Identity,
                         bias=nmr[:], scale=rstd[:])


def _transpose_to(nc, tp_ps, id_bf, src_bf, dst, dst_col0):
    """PE-transpose [P, C] bf16 tile into feature-major dst tiles.

    Evacuation copies alternate scalar/vector to balance engine load."""
    for c in range(NC_T):
        pt = tp_ps.tile([P, P], BF16, tag="tp", name="tp")
        nc.tensor.transpose(pt[:], src_bf[:, c * P:(c + 1) * P], id_bf)
        if c % 2 == 0:
            nc.scalar.copy(out=dst[c][:, dst_col0:dst_col0 + P], in_=pt[:])
        else:
            nc.vector.tensor_copy(out=dst[c][:, dst_col0:dst_col0 + P], in_=pt[:])


def build_nc():
    nc = bacc.Bacc("TRN2", target_bir_lowering=False, debug=False,
                   num_devices=N_CORES)

    x_seq = nc.dram_tensor("x_seq", [T, C], F32, kind="ExternalInput").ap()
    x_own = nc.dram_tensor("x_own", [CHUNK, C], F32, kind="ExternalInput").ap()
    wqkv = nc.dram_tensor("wqkv", [C, WQKV_W], BF16, kind="ExternalInput").ap()
    bqk = nc.dram_tensor("bqk", [P, 4], F32, kind="ExternalInput").ap()
    bv = nc.dram_tensor("bv", [1, VW], F32, kind="ExternalInput").ap()
    wproj = nc.dram_tensor("wproj", [2 * P, C], BF16, kind="ExternalInput").ap()
    wfc = nc.dram_tensor("wfc", [C, FF], BF16, kind="ExternalInput").ap()
    wmproj = nc.dram_tensor("wmproj", [FF, C], BF16, kind="ExternalInput").ap()
    bfc = nc.dram_tensor("bfc", [FF // P, P, 1], F32, kind="ExternalInput").ap()
    trimask = nc.dram_tensor("trimask", [P, P], BF16, kind="ExternalInput").ap()
    out = nc.dram_tensor("out", [CHUNK, C], F32, kind="ExternalOutput").ap()

    with tile.TileContext(nc) as tc:
        _body(nc, tc, x_seq, x_own, wqkv, bqk, bv, wproj,
              wfc, wmproj, bfc, trimask, out)
    nc.compile()
    return nc


def _body(nc, tc, x_seq, x_own, wqkv, bqk, bv, wproj,
          wfc, wmproj, bfc, trimask, out):
    es = ExitStack()
    with es:
        persist = es.enter_context(tc.tile_pool(name="persist", bufs=1))
        stats_pool = es.enter_context(tc.tile_pool(name="stats", bufs=4))
        cc_pool = es.enter_context(tc.tile_pool(name="cc", bufs=1,
                                                space="DRAM"))

        id_bf = persist.tile([P, P], BF16, tag="id", name="id")
        make_identity(nc, id_bf)
        eps_t = persist.tile([P, 1], F32, tag="eps", name="eps")
        nc.vector.memset(eps_t, EPS)
        bv_bc = persist.tile([P, VW], F32, tag="bv_bc", name="bv_bc")
        nc.gpsimd.dma_start(out=bv_bc, in_=bv.to_broadcast([P, VW]))
        tri = persist.tile([P, 1, P], BF16, tag="tri", name="tri")
        nc.sync.dma_start(out=tri,
                          in_=trimask.rearrange("k (one q) -> k one q", one=1))
        # own-stripe residual x2 (post-attention), f32, one per stripe
        xo2 = [persist.tile([P, C], F32, tag=f"xo2{t}", name=f"xo2{t}")
               for t in range(NT_OWN)]
        # LN2 outputs + prefetched RS results / own-stripe x
        h2b = [persist.tile([P, C], BF16, tag=f"h2b{t}", name=f"h2b{t}")
               for t in range(NT_OWN)]
        ccp = [persist.tile([P, C], BF16, tag=f"ccp{t}", name=f"ccp{t}")
               for t in range(NT_OWN)]
        xop = [persist.tile([P, C], F32, tag=f"xop{t}", name=f"xop{t}")
               for t in range(NT_OWN)]

        # cproj partial exchange buffers, one pair per 512-token chunk
        cc_in = [cc_pool.tile([512, C], BF16, tag=f"cci{i}", name=f"cci{i}")
                 for i in range(NQC)]
        cc_out = [cc_pool.tile([P, C], BF16, tag=f"cco{i}", name=f"cco{i}")
                  for i in range(NQC)]

        # persistent QKV outputs + weights
        qkv_cm = tc.tile_pool(name="qkv_out", bufs=1)
        qkv_out = qkv_cm.__enter__()
        qT = [qkv_out.tile([P, T], BF16, tag=f"qT{f}", name=f"qT{f}")
              for f in range(2)]
        kT = [qkv_out.tile([P, T], BF16, tag=f"kT{f}", name=f"kT{f}")
              for f in range(2)]
        Vt = [qkv_out.tile([P, HL, HD + 1], BF16, tag=f"Vt{t}", name=f"Vt{t}")
              for t in range(NT_SEQ)]
        wp_sb = [qkv_out.tile([P, C], BF16, tag=f"wp{f}", name=f"wp{f}")
                 for f in range(2)]
        h2T = [qkv_out.tile([P, CHUNK], BF16, tag=f"h2T{c}", name=f"h2T{c}")
               for c in range(NC_T)]
        bfc_all = qkv_out.tile([P, FF // P], F32, tag="bfc", name="bfc_all")
        nc.gpsimd.dma_start(out=bfc_all[:],
                            in_=bfc.rearrange("f p one -> p (f one)"))

        # =========== Phase A: LN1 + transposes + V + Q/K, interleaved =======
        with tc.tile_pool(name="ph_a", bufs=3) as ph_a, \
             tc.tile_pool(name="hT_p", bufs=1) as hT_p, \
             tc.tile_pool(name="tp_ps", bufs=3, space="PSUM") as tp_ps, \
             tc.tile_pool(name="mm_ps", bufs=5, space="PSUM") as mm_ps, \
             tc.tile_pool(name="w_p", bufs=1) as w_p:

            hT = [hT_p.tile([P, T], BF16, tag=f"hT{c}", name=f"hT{c}")
                  for c in range(NC_T)]
            # one big DMA for all QKV weights; gpsimd queue keeps both the
            # sync queue (x tiles) and the scalar queue (LN applies) clear
            wqkv_sb = w_p.tile([P, NC_T, WQKV_W], BF16, tag="wqkv",
                               name="wqkv")
            nc.gpsimd.dma_start(out=wqkv_sb,
                                in_=wqkv.rearrange("(c p) w -> p c w", p=P))
            bqk_sb = w_p.tile([P, 4], F32, tag="bqk", name="bqk_sb")
            nc.gpsimd.dma_start(out=bqk_sb, in_=bqk)
            for f2 in range(2):
                nc.gpsimd.dma_start(out=wp_sb[f2],
                                    in_=wproj[f2 * P:(f2 + 1) * P, :])

            x16 = x_seq.rearrange("(t p) c -> p t c", p=P)
            with nc.named_scope("ln1_qkv"):
                for g in range(NT_SEQ // 2):
                    xb = ph_a.tile([P, 2, C], F32, tag="xb", name="xb")
                    nc.sync.dma_start(out=xb, in_=x16[:, 2 * g:2 * g + 2, :])
                    for j in range(2):
                        t = 2 * g + j
                        h_bf = ph_a.tile([P, C], BF16, tag="h_bf", name="h_bf")
                        _ln_tile(nc, stats_pool, xb[:, j, :], h_bf, eps_t)
                        _transpose_to(nc, tp_ps, id_bf, h_bf, hT, t * P)
                        # V for this token tile (token-major, keys on parts)
                        ps = mm_ps.tile([P, VW], F32, tag="mm", name="mm")
                        for ci, c in enumerate(reversed(range(NC_T))):
                            nc.tensor.matmul(
                                ps[:], hT[c][:, t * P:(t + 1) * P],
                                wqkv_sb[:, c, 2 * HL * HD:WQKV_W],
                                start=(ci == 0), stop=(ci == NC_T - 1))
                        nc.vector.tensor_add(
                            Vt[t][:],
                            ps.rearrange("p (h e) -> p h e", e=HD + 1),
                            bv_bc.rearrange("p (h e) -> p h e", e=HD + 1))
                        if t % 4 == 3:
                            qc = t // 4
                            for f2 in range(2):
                                for qk in range(2):  # 0 = Q, 1 = K
                                    psq = mm_ps.tile([P, 512], F32, tag="mm",
                                                     name="mm")
                                    col = qk * P * 2 + f2 * P
                                    for ci, c in enumerate(
                                            reversed(range(NC_T))):
                                        nc.tensor.matmul(
                                            psq[:],
                                            wqkv_sb[:, c, col:col + P],
                                            hT[c][:, qc * 512:(qc + 1) * 512],
                                            start=(ci == 0),
                                            stop=(ci == NC_T - 1))
                                    dst = qT if qk == 0 else kT
                                    # bias is per-partition (feature-major):
                                    # evacuate on ACT to keep DVE clear
                                    nc.scalar.activation(
                                        dst[f2][:, qc * 512:(qc + 1) * 512],
                                        psq[:], AF.Identity,
                                        bias=bqk_sb[:, 2 * qk + f2:
                                                    2 * qk + f2 + 1],
                                        scale=1.0)

        # fc weights resident: pool opened after phase A (reuses hT's SBUF),
        # loads emitted now on the gpsimd queue, consumed in phase C.
        wfc_cm = tc.tile_pool(name="wfc_p", bufs=1)
        wfc_p = wfc_cm.__enter__()
        wfc_sb = [wfc_p.tile([P, NC_T, 512], BF16, tag=f"wf{g}",
                             name=f"wf{g}") for g in range(FF // 512)]
        wfc4 = wfc.rearrange("(c p) f -> p c f", p=P)
        for g8 in range(FF // 512):
            nc.gpsimd.dma_start(out=wfc_sb[g8],
                                in_=wfc4[:, :, g8 * 512:(g8 + 1) * 512])

        # =========== Phase B: attention (4 local heads, exact causal) =======
        scale = 1.0 / math.sqrt(HD)
        with tc.tile_pool(name="sc_ps", bufs=2, space="PSUM") as sc_ps, \
             tc.tile_pool(name="y_ps", bufs=2, space="PSUM") as y_ps, \
             tc.tile_pool(name="c_ps", bufs=2, space="PSUM") as c_ps, \
             tc.tile_pool(name="p_sb", bufs=4) as p_sb, \
             tc.tile_pool(name="yT2_p", bufs=4) as yT2_p, \
             tc.tile_pool(name="cp_sb", bufs=3) as cp_sb, \
             tc.tile_pool(name="rec_p", bufs=4) as rec_p:

            # prefetch own-stripe x (no RS dependency)
            for s in range(NT_OWN):
                nc.sync.dma_start(out=xop[s][:],
                                  in_=x_own[s * P:(s + 1) * P, :])

            with nc.named_scope("attention"):
                for qc in range(NQC):
                    nkt = 4 * qc + 4
                    yT2_f = [yT2_p.tile([P, 512], BF16, tag=f"y{f}",
                                        name=f"y{f}") for f in range(2)]
                    for f2 in range(2):
                        yps = [y_ps.tile([P, 512], F32, tag="y", name="y")
                               for _ in range(2)]

                        def _pv(piece):
                            kt, pt, q0, lp, cs = piece
                            for hsub in range(2):
                                h = 2 * f2 + hsub
                                nc.tensor.matmul(
                                    yps[hsub][0:HD + 1, cs:cs + lp],
                                    Vt[kt][:, h, :], pt[:, hsub, 0:lp],
                                    start=(kt == 0), stop=(kt == nkt - 1))

                        pieces = []
                        for kt in range(nkt):
                            q0 = max(kt * P, qc * 512)
                            lp = (qc + 1) * 512 - q0
                            cs = q0 - qc * 512
                            ps = sc_ps.tile([P, 2, 512], F32, tag="sc",
                                            name="sc")
                            for hsub in range(2):
                                ho = hsub * HD
                                nc.tensor.matmul(
                                    ps[:, hsub, 0:lp],
                                    kT[f2][ho:ho + HD, kt * P:(kt + 1) * P],
                                    qT[f2][ho:ho + HD, q0:q0 + lp],
                                    start=True, stop=True,
                                    tile_position=(ho, 0))
                            pt = p_sb.tile([P, 2, 512], BF16, tag="P",
                                           name="Pt")
                            nc.scalar.activation(pt[:, :, 0:lp],
                                                 ps[:, :, 0:lp],
                                                 AF.Exp, scale=scale)
                            if q0 == kt * P:
                                nc.vector.tensor_mul(
                                    pt[:, :, 0:P], pt[:, :, 0:P],
                                    tri[:, 0:1, :].broadcast_to([P, 2, P]))
                            pieces.append((kt, pt, q0, lp, cs))
                            if len(pieces) >= 2:
                                _pv(pieces[-2])
                        _pv(pieces[-1])

                        # normalize: copy denom row -> bcast to 64 lanes ->
                        # reciprocal (64-wide; a single-lane [1,512] recip
                        # costs 3.3us) -> mul. The odd head's result is
                        # partition-shifted via DMA (matmul out base must be
                        # 0/32/64, so its y cannot land at 64:128 directly).
                        for hsub in range(2):
                            s_t = rec_p.tile([1, 512], F32, tag="s", name="s")
                            nc.vector.tensor_copy(
                                out=s_t[:], in_=yps[hsub][HD:HD + 1, :])
                            # 1/d = exp(-ln d) on ACT: DVE reciprocal is
                            # ~6.4ns/elem/lane (3.3us for 512 elems on one
                            # lane) and ACT Reciprocal is banned; d >= 1
                            # always (diagonal contributes exp(0)=1)
                            nc.scalar.activation(s_t[:], s_t[:], AF.Ln)
                            nc.scalar.activation(s_t[:], s_t[:], AF.Exp,
                                                 scale=-1.0)
                            rb = rec_p.tile([HD, 512], F32, tag="rb",
                                            name="rb")
                            nc.gpsimd.partition_broadcast(rb[:], s_t[:])
                            if hsub == 0:
                                nc.vector.tensor_mul(
                                    yT2_f[f2][0:HD, :], yps[0][0:HD, :],
                                    rb[:])
                            else:
                                ytmp = rec_p.tile([HD, 512], BF16, tag="yt",
                                                  name="yt")
                                nc.vector.tensor_mul(
                                    ytmp[:], yps[1][0:HD, :], rb[:])
                                nc.sync.dma_start(
                                    out=yT2_f[f2][HD:2 * HD, :], in_=ytmp[:])

                    # ---- cproj partials + this chunk's ReduceScatter ----
                    for tt in range(4):
                        cpart = cp_sb.tile([P, C], BF16, tag="cp", name="cp")
                        for n in range(2):
                            psn = c_ps.tile([P, 512], F32, tag="c", name="c")
                            for f2 in range(2):
                                nc.tensor.matmul(
                                    psn[:],
                                    yT2_f[f2][:, tt * P:(tt + 1) * P],
                                    wp_sb[f2][:, n * 512:(n + 1) * 512],
                                    start=(f2 == 0), stop=(f2 == 1))
                            nc.vector.tensor_copy(
                                out=cpart[:, n * 512:(n + 1) * 512],
                                in_=psn[:])
                        nc.sync.dma_start(
                            out=cc_in[qc][tt * P:(tt + 1) * P, :],
                            in_=cpart[:])
                    nc.gpsimd.collective_compute(
                        "ReduceScatter", mybir.AluOpType.add,
                        replica_groups=REPLICA_GROUPS,
                        ins=[cc_in[qc][:]], outs=[cc_out[qc][:]])
                    # prefetch RS results with a 2-chunk lag so the RS-gated
                    # DMA never parks the in-order sync queue on a collective
                    if qc >= 2:
                        nc.sync.dma_start(out=ccp[qc - 2][:],
                                          in_=cc_out[qc - 2][:])
                    if qc == NQC - 1:
                        nc.sync.dma_start(out=ccp[qc - 1][:],
                                          in_=cc_out[qc - 1][:])
                        nc.sync.dma_start(out=ccp[qc][:],
                                          in_=cc_out[qc][:])

        # =========== Phase C: transposes + fc + mproj, per stripe pair =====
        with tc.tile_pool(name="tp_ps2", bufs=2, space="PSUM") as tp_ps2, \
             tc.tile_pool(name="f_ps", bufs=2, space="PSUM") as f_ps, \
             tc.tile_pool(name="m_ps", bufs=4, space="PSUM") as m_ps, \
             tc.tile_pool(name="aT_p", bufs=1) as aT_p, \
             tc.tile_pool(name="wm_str", bufs=3) as wm_str, \
             tc.tile_pool(name="out_p", bufs=2) as out_p:

            aT = [aT_p.tile([P, 256], BF16, tag=f"aT{f}", name=f"aT{f}")
                  for f in range(FF // P)]

            def _fc(pair):
                scol = pair * 256
                for g8 in range(FF // 512):
                    for i in range(4):
                        ffi = g8 * 4 + i
                        psf = f_ps.tile([P, 256], F32, tag="f", name="f")
                        for ci, c in enumerate(range(NC_T)):
                            nc.tensor.matmul(
                                psf[:],
                                wfc_sb[g8][:, c, i * P:(i + 1) * P],
                                h2T[c][:, scol:scol + 256],
                                start=(ci == 0), stop=(ci == NC_T - 1))
                        nc.scalar.activation(
                            aT[ffi][:], psf[:], AF.Gelu_apprx_tanh,
                            bias=bfc_all[:, ffi:ffi + 1])

            def _mproj_out(pair):
                stripes = [2 * pair, 2 * pair + 1]
                psm = [m_ps.tile([P, 512], F32, tag="m", name="m")
                       for _ in range(4)]
                for f in range(FF // P):
                    wmb = wm_str.tile([P, C], BF16, tag="wm", name="wm")
                    nc.gpsimd.dma_start(
                        out=wmb, in_=wmproj[f * P:(f + 1) * P, :])
                    for si in range(2):
                        for n in range(2):
                            nc.tensor.matmul(
                                psm[2 * si + n][:],
                                aT[f][:, si * P:(si + 1) * P],
                                wmb[:, n * 512:(n + 1) * 512],
                                start=(f == 0), stop=(f == FF // P - 1))
                for si, s in enumerate(stripes):
                    o_t = out_p.tile([P, C], F32, tag="o", name="o")
                    for n in range(2):
                        nc.vector.tensor_add(
                            o_t[:, n * 512:(n + 1) * 512],
                            psm[2 * si + n][:],
                            xo2[s][:, n * 512:(n + 1) * 512])
                    nc.sync.dma_start(out=out[s * P:(s + 1) * P, :],
                                      in_=o_t[:])

            def _ln2(s):
                nc.vector.tensor_add(xo2[s][:], ccp[s][:], xop[s][:])
                _ln_tile(nc, stats_pool, xo2[s], h2b[s], eps_t)

            with nc.named_scope("mlp"):
                for s in (0, 1):
                    _ln2(s)
                for s in (0, 1):
                    _transpose_to(nc, tp_ps2, id_bf, h2b[s], h2T, s * P)
                _fc(0)
                for s in (2, 3):
                    _ln2(s)
                _mproj_out(0)
                # pair-1 transposes after mproj{01}: RS(3) + its LN2 finish
                # well within fc{01}+mproj{01}
                for s in (2, 3):
                    _transpose_to(nc, tp_ps2, id_bf, h2b[s], h2T, s * P)
                _fc(1)
                _mproj_out(1)

        wfc_cm.__exit__(None, None, None)
        qkv_cm.__exit__(None, None, None)


# ---------------------------------------------------------------------------
# Host side
# ---------------------------------------------------------------------------

_NC_CACHE = {}


def _get_nc():
    if "nc" not in _NC_CACHE:
        _NC_CACHE["nc"] = build_nc()
    return _NC_CACHE["nc"]


def make_in_maps(x, w_attn, w_attn_proj, w_fc, w_mlp_proj,
                 ln1_g, ln1_b, ln2_g, ln2_b):
    bf = ml_dtypes.bfloat16
    f32 = np.float32
    x = np.asarray(x, f32)
    w_attn = np.asarray(w_attn, f32)
    ln1_g = np.asarray(ln1_g, f32)
    ln1_b = np.asarray(ln1_b, f32)
    ln2_g = np.asarray(ln2_g, f32)
    ln2_b = np.asarray(ln2_b, f32)

    w1 = ln1_g[:, None] * w_attn
    b1 = ln1_b @ w_attn              # [3C]
    wfc_h = (ln2_g[:, None] * np.asarray(w_fc, f32)).astype(bf)
    bfc_h = (ln2_b @ np.asarray(w_fc, f32)).astype(f32).reshape(FF // P, P, 1)
    wmproj_h = np.asarray(w_mlp_proj, f32).astype(bf)
    wproj_full = np.asarray(w_attn_proj, f32).reshape(H, HD, C)

    tri_h = (np.arange(P)[:, None] <= np.arange(P)[None, :]).astype(bf)

    in_maps = []
    for core in range(N_CORES):
        b, r = divmod(core, GRP)
        h0 = HL * r
        wq_h = w1[:, h0 * HD:(h0 + HL) * HD]
        wk_h = w1[:, C + h0 * HD:C + (h0 + HL) * HD]
        wv_h = np.zeros((C, VW), f32)
        bv_h = np.zeros((1, VW), f32)
        for hl in range(HL):
            h = h0 + hl
            base = hl * (HD + 1)
            wcol = 2 * C + h * HD
            wv_h[:, base:base + HD] = w1[:, wcol:wcol + HD]
            bv_h[0, base:base + HD] = b1[wcol:wcol + HD]
            bv_h[0, base + HD] = 1.0    # ones column -> softmax denominator
        wqkv_h = np.concatenate([wq_h, wk_h, wv_h], axis=1).astype(bf)
        bq_h = b1[h0 * HD:(h0 + HL) * HD].reshape(2, P).T
        bk_h = b1[C + h0 * HD:C + (h0 + HL) * HD].reshape(2, P).T
        bqk_h = np.concatenate([bq_h, bk_h], axis=1)
        wproj_h = np.ascontiguousarray(
            wproj_full[h0:h0 + HL].reshape(HL * HD, C)).astype(bf)
        # stripe i of core r = tokens [i*512 + r*128, i*512 + r*128 + 128)
        x_own_h = np.concatenate(
            [x[b, i * 512 + r * P:i * 512 + r * P + P] for i in range(NQC)])
        in_maps.append({
            "x_seq": np.ascontiguousarray(x[b]),
            "x_own": np.ascontiguousarray(x_own_h),
            "wqkv": np.ascontiguousarray(wqkv_h),
            "bqk": np.ascontiguousarray(bqk_h, f32),
            "bv": bv_h,
            "wproj": wproj_h, "wfc": wfc_h, "bfc": bfc_h,
            "wmproj": wmproj_h, "trimask": tri_h,
        })
    return in_maps


def kernel(x, w_attn, w_attn_proj, w_fc, w_mlp_proj,
           ln1_g, ln1_b, ln2_g, ln2_b):
    from concourse.bass_utils import run_bass_kernel_spmd
    nc = _get_nc()
    in_maps = make_in_maps(x, w_attn, w_attn_proj, w_fc, w_mlp_proj,
                           ln1_g, ln1_b, ln2_g, ln2_b)
    res = run_bass_kernel_spmd(nc, in_maps, core_ids=list(range(N_CORES)))
    out = np.empty((B, T, C), np.float32)
    for core in range(N_CORES):
        b, r = divmod(core, GRP)
        for i in range(NQC):
            out[b, i * 512 + r * P:i * 512 + r * P + P] = \
                res.results[core]["out"][i * P:(i + 1) * P]
    return out
